# revision 26
# baseline (speedup 1.0000x reference)
"""RGCN-BDD link-predict layer kernel for 8 TRN2 NeuronCores.

Strategy: shard edges by destination-node slice (6250 nodes/device) so the
segment-sum is fully local; run the two RGCN layers as two launches of one
compiled single-layer NEFF, with host-side ReLU/bias between launches.

Per device, per layer (fused single pass):
  - edges are dst-sorted; per 128-node chunk the relevant edge tiles form a
    monotone sliding window, so per-edge product tiles stay SBUF-resident.
  - per 128-edge tile: load src features in (i,b)-permuted layout (bf16);
    indirect-gather per-edge block-diagonal weight rows from a host-permuted
    (i,j,b)-layout fp8_e3m4 table, cast to bf16 during the gather; a single
    full-width DVE multiply with a stride-0 broadcast view of the features
    forms all 2500 partial products (no on-chip expansion); DVE pairwise
    adds fold i-slices 0..3 into one.
  - per chunk: segment-sum via tensor-engine matmuls with host-built fp8
    one-hot matrices (entries carry the scaled edge norm), accumulated in
    PSUM together with the self-loop matmul; ACT copies PSUM out with the
    dequant scale.
"""
import sys
if '/opt/trn_rl_repo' not in sys.path:
    sys.path.insert(0, '/opt/trn_rl_repo')

import numpy as np
import ml_dtypes

import concourse.bass as bass
import concourse.bacc as bacc
import concourse.mybir as mybir
import concourse.tile as tile
from concourse.bass_utils import run_bass_kernel_spmd

# problem constants (hardcoded per spec)
NN = 50000      # num nodes
H = 500         # hidden dim
NB = 100        # num bases
SUB = 5         # block size
W_COLS = NB * SUB * SUB  # 2500
NR2 = 474       # num relations * 2
E = 100000      # num edges
NDEV = 8
P = 128
NPD = NN // NDEV          # 6250 nodes per device
NCH = (NPD + P - 1) // P  # 49 chunks
N_PAD = NCH * P           # 6272
KQ4 = 512  # K padded to 4*128 (zero rows beyond 500)

SW = 16.0   # weight-table fp8 scale
SN = 16.0   # one-hot (norm) fp8 scale
F8MAX = 15.5
# fold config: number of pairwise i-slice adds on DVE. NADD=3 folds slices
# 0..3 into one (tree t0=s0+s1, t1=s2+s3, u=t0+t1), leaving NMM=2 rhs
# slices {u, s4} for the tensor engine.
NADD = 3
NMM = 5 - NADD if NADD < 4 else 1
GP_T1 = False  # gpsimd folds collide with SWDGE descriptor generation

BF = mybir.dt.bfloat16
F8 = mybir.dt.float8e3
F32 = mybir.dt.float32
I32 = mybir.dt.int32

_cache = {}


def _plan(src, dst, etype, norm):
    """Host-side sharding plan; layer-invariant."""
    src = np.asarray(src).astype(np.int64)
    dst = np.asarray(dst).astype(np.int64)
    etype = np.asarray(etype).astype(np.int64)
    norm = np.asarray(norm).astype(np.float32).reshape(-1)

    dev_of = dst // NPD
    per = []
    for d in range(NDEV):
        sel = np.nonzero(dev_of == d)[0]
        dl = dst[sel] - d * NPD
        order = np.argsort(dl, kind='stable')
        el = sel[order]
        per.append((el, dl[order]))
    n_max = max(len(el) for el, _ in per)
    ET = (n_max + P - 1) // P

    # per-device padded src index list (for host-side pre-gather of xe rows)
    srcl = np.zeros((NDEV, ET * P), np.int64)

    # per-chunk union windows over edge tiles (same for all devices)
    W0 = np.zeros(NCH, np.int64)
    WEND = np.zeros(NCH, np.int64)
    for c in range(NCH):
        lo, hi = [], []
        for el, dl in per:
            e0 = np.searchsorted(dl, c * P, 'left')
            e1 = np.searchsorted(dl, (c + 1) * P, 'left')
            lo.append(e0 // P)
            hi.append((e1 + P - 1) // P if e1 > 0 else 0)
        W0[c] = min(lo)
        WEND[c] = max(max(hi), W0[c] + 1)
    WEND = np.minimum(WEND, ET)
    W0 = np.minimum(W0, WEND - 1)
    KE = (WEND - W0).astype(np.int64)
    OHT = int(KE.sum())           # total one-hot tiles
    ohoff = np.concatenate([[0], np.cumsum(KE)])[:NCH].astype(np.int64)

    # per-device static input arrays. oh is stored partition-major per chunk:
    # row ohoff[c]*P + p*KE[c] + kk holds edge-slot p of window tile kk, so
    # the per-chunk load is contiguous per partition.
    etn = np.zeros((NDEV, P, ET), np.int32)
    oh = np.zeros((NDEV, OHT * P, P), np.float32)
    for d in range(NDEV):
        el, dl = per[d]
        n_d = len(el)
        pad = ET * P - n_d
        srcl[d] = np.pad(src[el], (0, pad))
        etn[d] = np.pad(etype[el], (0, pad)).astype(np.int32).reshape(ET, P).T
        nr = norm[el]
        for c in range(NCH):
            for kk in range(KE[c]):
                g0 = (W0[c] + kk) * P
                rows = np.arange(g0, g0 + P)
                valid = rows < n_d
                m = dl[rows[valid]] - c * P
                ok = (m >= 0) & (m < P)
                j = np.nonzero(valid)[0][ok]
                oh[d, ohoff[c] * P + j * KE[c] + kk, m[ok]] = nr[rows[valid]][ok]

    oh8 = np.clip(oh * SN, -F8MAX, F8MAX).astype(ml_dtypes.float8_e3m4)
    return dict(ET=ET, srcl=srcl, etn=etn,
                oh=oh8, W0=W0, KE=KE, ohoff=ohoff, OHT=OHT)


def _build_nc(ET, W0, KE, ohoff, OHT):
    nc = bacc.Bacc(None, target_bir_lowering=False)

    xs = nc.dram_tensor("xs", [ET // 2 * P, 2 * H], BF, kind="ExternalInput")
    xtp = nc.dram_tensor("xtp", [NCH * P, 4 * P], BF, kind="ExternalInput")
    wf = nc.dram_tensor("wf", [NR2, W_COLS], F8, kind="ExternalInput")
    lw = nc.dram_tensor("lw", [KQ4, H], BF, kind="ExternalInput")
    etn = nc.dram_tensor("etn", [P, ET], I32, kind="ExternalInput")
    oh = nc.dram_tensor("oh", [OHT * P, P], F8, kind="ExternalInput")
    out = nc.dram_tensor("out", [N_PAD, H], BF, kind="ExternalOutput")

    with tile.TileContext(nc) as tc:
        with tc.tile_pool(name="const", bufs=1) as constp, \
             tc.tile_pool(name="s1", bufs=4) as s1, \
             tc.tile_pool(name="prodp", bufs=8) as prodp, \
             tc.tile_pool(name="t01p", bufs=6) as t01p, \
             tc.tile_pool(name="s2", bufs=4) as s2, \
             tc.tile_pool(name="psum", bufs=6, space="PSUM") as psp:

            # load the gather indices first (they gate the whole pipeline),
            # then the loop weights
            etn_sb = constp.tile([P, ET], I32, tag="etn")
            nc.sync.dma_start(out=etn_sb[:], in_=etn[:, :])
            lw_sb = []
            for q in range(4):
                t = constp.tile([P, H], BF, tag=f"lw{q}")
                nc.sync.dma_start(out=t[:], in_=lw[q * 128:(q + 1) * 128, :])
                lw_sb.append(t)

            prods = {}   # edge-tile idx -> list of NMM rhs views (+ tiles)

            def produce_pair(t):
                """Process edge tiles t and t+1 with pair-merged DVE ops."""
                xe2 = s1.tile([P, 2 * H], BF, tag="xe2")
                wg2 = s1.tile([P, 2 * W_COLS], BF, tag="wg2")
                m = t // 2
                nc.scalar.dma_start(out=xe2[:], in_=xs[m * P:(m + 1) * P, :])
                for k in range(2):
                    nc.gpsimd.indirect_dma_start(
                        out=wg2[:, k * W_COLS:(k + 1) * W_COLS],
                        out_offset=None, in_=wf[:, :],
                        in_offset=bass.IndirectOffsetOnAxis(
                            ap=etn_sb[:, t + k:t + k + 1], axis=0))
                # one full-width multiply for both tiles via the merged (k,i)
                # dim and a stride-0 broadcast over j:
                # prod[p, (k,i), j, b] = xe2[p, (k,i,b)] * wg2[p, (k,i,j,b)]
                pr2 = prodp.tile([P, 2 * W_COLS], BF, tag="pr2")
                xe_b = xe2[:].rearrange("p (ki u b) -> p ki u b", ki=2 * SUB, u=1) \
                             .to_broadcast([P, 2 * SUB, SUB, NB])
                nc.vector.tensor_tensor(
                    out=pr2[:].rearrange("p (ki j b) -> p ki j b", ki=2 * SUB, j=SUB),
                    in0=xe_b,
                    in1=wg2[:].rearrange("p (ki j b) -> p ki j b", ki=2 * SUB, j=SUB),
                    op=mybir.AluOpType.mult)
                # pair folds in two ops: one TT computes t0=s0+s1 and
                # t1=s2+s3 for both tiles via a [p, k, {0,2}, f] view; the
                # second folds t0+t1.
                pv = pr2[:].rearrange("p (k i f) -> p k i f", k=2, i=SUB)
                ev = pv[:, :, 0:4:2, :]   # slices i=0,2
                ov = pv[:, :, 1:4:2, :]   # slices i=1,3
                t01 = t01p.tile([P, 4 * H], BF, tag="t01")
                t01v = t01[:].rearrange("p (k i f) -> p k i f", k=2, i=2)
                nc.vector.tensor_tensor(out=t01v, in0=ev, in1=ov,
                                        op=mybir.AluOpType.add)
                t4 = t01[:].rearrange("p (k i f) -> p k i f", k=2, i=2)
                if (t // 2) % 3 == 2:
                    # skip the up-fold; feed t0, t1, s4 to the tensor engine
                    for k in range(2):
                        s4 = pr2[:, (k * SUB + 4) * H:(k * SUB + 5) * H]
                        prods[t + k] = [t01[:, (2 * k) * H:(2 * k + 1) * H],
                                        t01[:, (2 * k + 1) * H:(2 * k + 2) * H],
                                        s4]
                else:
                    up = prodp.tile([P, 2 * H], BF, tag="up")
                    nc.vector.tensor_tensor(
                        out=up[:].rearrange("p (k f) -> p k f", k=2),
                        in0=t4[:, :, 0, :], in1=t4[:, :, 1, :],
                        op=mybir.AluOpType.add)
                    for k in range(2):
                        s4 = pr2[:, (k * SUB + 4) * H:(k * SUB + 5) * H]
                        prods[t + k] = [up[:, k * H:(k + 1) * H], s4]

            produced = 0
            for c in range(NCH):
                need = int(W0[c] + KE[c])
                while produced < need:
                    produce_pair(produced)
                    produced += 2
                ps = psp.tile([P, H], F32, tag="ps")
                ke = int(KE[c])
                ohsb = s2.tile([P, 7 * P], F8, tag="ohsb")
                o0 = int(ohoff[c]) * P
                nc.sync.dma_start(
                    out=ohsb[:, :ke * P],
                    in_=oh[o0:o0 + ke * P, :].rearrange("(p k) m -> p (k m)", p=P))
                xt = s2.tile([P, 4 * P], BF, tag="xt")
                nc.sync.dma_start(out=xt[:], in_=xtp[c * P:(c + 1) * P, :])
                first = True
                for kk in range(ke):
                    t = int(W0[c]) + kk
                    for rv in prods[t]:
                        nc.tensor.matmul(out=ps[:],
                                         lhsT=ohsb[:, kk * P:(kk + 1) * P],
                                         rhs=rv, start=first, stop=False)
                        first = False
                for q in range(4):
                    nc.tensor.matmul(out=ps[:], lhsT=xt[:, q * P:(q + 1) * P],
                                     rhs=lw_sb[q][:],
                                     start=False, stop=(q == 3))
                # PSUM columns are in (j,b) order; host un-permutes
                outt = s2.tile([P, H], BF, tag="outt")
                nc.scalar.activation(out=outt[:], in_=ps[:],
                                     func=mybir.ActivationFunctionType.Copy,
                                     scale=1.0 / (SW * SN))
                nc.sync.dma_start(out=out[c * P:(c + 1) * P, :], in_=outt[:])
                # drop window tiles no longer needed
                if c + 1 < NCH:
                    for t in [k for k in prods if k < int(W0[c + 1])]:
                        del prods[t]
    nc.finalize()
    return nc


def _run_layer(nc, plan, xp, xb, wfp, lwb, trace=False):
    """One RGCN-BDD layer (pre-bias, pre-activation) on 8 cores.

    xp: (i,b)-permuted features bf16 [NN, H]; xb: raw features bf16 [NN, H].
    """
    ET = plan['ET']
    in_maps = []
    for d in range(NDEV):
        # pair-swizzled per-edge features: row m*P+p holds tiles (2m, 2m+1)
        xg = xp[plan['srcl'][d]]
        xsd = np.ascontiguousarray(
            xg.reshape(ET // 2, 2, P, H).transpose(0, 2, 1, 3)
              .reshape(ET // 2 * P, 2 * H))
        # per-chunk contiguous transposed features: row c*P+p col q*P+m
        xt_full = np.zeros((KQ4, N_PAD), ml_dtypes.bfloat16)
        xt_full[:H, :NPD] = xb[d * NPD:(d + 1) * NPD].T
        xtpd = np.ascontiguousarray(
            xt_full.reshape(4, P, NCH, P).transpose(2, 1, 0, 3)
                   .reshape(NCH * P, 4 * P))
        in_maps.append({
            "xs": xsd, "xtp": xtpd, "wf": wfp, "lw": lwb,
            "etn": plan['etn'][d], "oh": plan['oh'][d],
        })
    res = run_bass_kernel_spmd(nc, in_maps, core_ids=list(range(NDEV)),
                               trace=trace)
    outp = np.empty((NN, H), np.float32)
    for d in range(NDEV):
        outp[d * NPD:(d + 1) * NPD] = res.results[d]["out"][:NPD].astype(np.float32)
    # device output columns are in (j,b) order; un-permute to (b,j)
    outp = np.ascontiguousarray(
        outp.reshape(NN, SUB, NB).transpose(0, 2, 1)).reshape(NN, H)
    return outp, res


def _pad_lw(lw):
    # permute output columns (b,j) -> (j,b) to match the message-path PSUM
    lwc = np.asarray(lw, np.float32).reshape(H, NB, SUB).transpose(0, 2, 1)
    lwp = np.zeros((KQ4, H), np.float32)
    lwp[:H] = lwc.reshape(H, H) * (SW * SN)
    return lwp.astype(ml_dtypes.bfloat16)


def _permute_w(W):
    # [r, b, i, j] -> [r, i, j, b] flattened, scaled, fp8_e3m4
    W = np.asarray(W, dtype=np.float32).reshape(NR2, NB, SUB, SUB)
    Wp = np.ascontiguousarray(
        W.transpose(0, 2, 3, 1).reshape(NR2, W_COLS)) * SW
    return np.clip(Wp, -F8MAX, F8MAX).astype(ml_dtypes.float8_e3m4)


def _permute_x(x):
    # [n, (b,i)] f32 -> [n, (i,b)] bf16
    return np.ascontiguousarray(
        x.reshape(-1, NB, SUB).transpose(0, 2, 1).reshape(-1, H)
    ).astype(ml_dtypes.bfloat16)


def kernel(nids, src, dst, etype, norm, emb, W1, loop_w1, bias1,
           W2, loop_w2, bias2, _trace=False, _times=None):
    key = "nc"
    if key not in _cache:
        plan = _plan(src, dst, etype, norm)
        nc = _build_nc(plan['ET'], plan['W0'], plan['KE'],
                       plan['ohoff'], plan['OHT'])
        _cache[key] = (plan, nc)
    plan, nc = _cache[key]

    x = np.asarray(emb, dtype=np.float32)[np.asarray(nids, dtype=np.int64)]
    h_pre, r1 = _run_layer(nc, plan, _permute_x(x),
                           x.astype(ml_dtypes.bfloat16),
                           _permute_w(W1), _pad_lw(loop_w1), trace=_trace)
    h = np.maximum(h_pre + np.asarray(bias1, dtype=np.float32)[None, :], 0.0)
    out_pre, r2 = _run_layer(nc, plan, _permute_x(h),
                             h.astype(ml_dtypes.bfloat16),
                             _permute_w(W2), _pad_lw(loop_w2), trace=_trace)
    out = out_pre + np.asarray(bias2, dtype=np.float32)[None, :]
    if _times is not None:
        _times.extend([r1, r2])
    return out


# revision 29
# speedup vs baseline: 1.0008x; 1.0008x over previous
"""RGCN-BDD link-predict layer kernel for 8 TRN2 NeuronCores.

Strategy: shard edges by destination-node slice (6250 nodes/device) so the
segment-sum is fully local; run the two RGCN layers as two launches of one
compiled single-layer NEFF, with host-side ReLU/bias between launches.

Per device, per layer (fused single pass):
  - edges are dst-sorted; per 128-node chunk the relevant edge tiles form a
    monotone sliding window, so per-edge product tiles stay SBUF-resident.
  - per 128-edge tile: load src features in (i,b)-permuted layout (bf16);
    indirect-gather per-edge block-diagonal weight rows from a host-permuted
    (i,j,b)-layout fp8_e3m4 table, cast to bf16 during the gather; a single
    full-width DVE multiply with a stride-0 broadcast view of the features
    forms all 2500 partial products (no on-chip expansion); DVE pairwise
    adds fold i-slices 0..3 into one.
  - per chunk: segment-sum via tensor-engine matmuls with host-built fp8
    one-hot matrices (entries carry the scaled edge norm), accumulated in
    PSUM together with the self-loop matmul; ACT copies PSUM out with the
    dequant scale.
"""
import sys
if '/opt/trn_rl_repo' not in sys.path:
    sys.path.insert(0, '/opt/trn_rl_repo')

import numpy as np
import ml_dtypes

import concourse.bass as bass
import concourse.bacc as bacc
import concourse.mybir as mybir
import concourse.tile as tile
from concourse.bass_utils import run_bass_kernel_spmd

# problem constants (hardcoded per spec)
NN = 50000      # num nodes
H = 500         # hidden dim
NB = 100        # num bases
SUB = 5         # block size
W_COLS = NB * SUB * SUB  # 2500
NR2 = 474       # num relations * 2
E = 100000      # num edges
NDEV = 8
P = 128
NPD = NN // NDEV          # 6250 nodes per device
NCH = (NPD + P - 1) // P  # 49 chunks
N_PAD = NCH * P           # 6272
KQ4 = 512  # K padded to 4*128 (zero rows beyond 500)

SW = 16.0   # weight-table fp8 scale
SN = 16.0   # one-hot (norm) fp8 scale
F8MAX = 15.5
# fold config: number of pairwise i-slice adds on DVE. NADD=3 folds slices
# 0..3 into one (tree t0=s0+s1, t1=s2+s3, u=t0+t1), leaving NMM=2 rhs
# slices {u, s4} for the tensor engine.
NADD = 3
NMM = 5 - NADD if NADD < 4 else 1
GP_T1 = False  # gpsimd folds collide with SWDGE descriptor generation

BF = mybir.dt.bfloat16
F8 = mybir.dt.float8e3
F32 = mybir.dt.float32
I32 = mybir.dt.int32

_cache = {}


def _plan(src, dst, etype, norm):
    """Host-side sharding plan; layer-invariant."""
    src = np.asarray(src).astype(np.int64)
    dst = np.asarray(dst).astype(np.int64)
    etype = np.asarray(etype).astype(np.int64)
    norm = np.asarray(norm).astype(np.float32).reshape(-1)

    dev_of = dst // NPD
    per = []
    for d in range(NDEV):
        sel = np.nonzero(dev_of == d)[0]
        dl = dst[sel] - d * NPD
        order = np.argsort(dl, kind='stable')
        el = sel[order]
        per.append((el, dl[order]))
    n_max = max(len(el) for el, _ in per)
    ET = (n_max + P - 1) // P

    # per-device padded src index list (for host-side pre-gather of xe rows)
    srcl = np.zeros((NDEV, ET * P), np.int64)

    # per-chunk union windows over edge tiles (same for all devices)
    W0 = np.zeros(NCH, np.int64)
    WEND = np.zeros(NCH, np.int64)
    for c in range(NCH):
        lo, hi = [], []
        for el, dl in per:
            e0 = np.searchsorted(dl, c * P, 'left')
            e1 = np.searchsorted(dl, (c + 1) * P, 'left')
            lo.append(e0 // P)
            hi.append((e1 + P - 1) // P if e1 > 0 else 0)
        W0[c] = min(lo)
        WEND[c] = max(max(hi), W0[c] + 1)
    WEND = np.minimum(WEND, ET)
    W0 = np.minimum(W0, WEND - 1)
    KE = (WEND - W0).astype(np.int64)
    OHT = int(KE.sum())           # total one-hot tiles
    ohoff = np.concatenate([[0], np.cumsum(KE)])[:NCH].astype(np.int64)

    # per-device static input arrays. oh is stored partition-major per chunk:
    # row ohoff[c]*P + p*KE[c] + kk holds edge-slot p of window tile kk, so
    # the per-chunk load is contiguous per partition.
    etn = np.zeros((NDEV, P, ET), np.int32)
    oh = np.zeros((NDEV, OHT * P, P), np.float32)
    for d in range(NDEV):
        el, dl = per[d]
        n_d = len(el)
        pad = ET * P - n_d
        srcl[d] = np.pad(src[el], (0, pad))
        etn[d] = np.pad(etype[el], (0, pad)).astype(np.int32).reshape(ET, P).T
        nr = norm[el]
        for c in range(NCH):
            for kk in range(KE[c]):
                g0 = (W0[c] + kk) * P
                rows = np.arange(g0, g0 + P)
                valid = rows < n_d
                m = dl[rows[valid]] - c * P
                ok = (m >= 0) & (m < P)
                j = np.nonzero(valid)[0][ok]
                oh[d, ohoff[c] * P + j * KE[c] + kk, m[ok]] = nr[rows[valid]][ok]

    oh8 = np.clip(oh * SN, -F8MAX, F8MAX).astype(ml_dtypes.float8_e3m4)
    return dict(ET=ET, srcl=srcl, etn=etn,
                oh=oh8, W0=W0, KE=KE, ohoff=ohoff, OHT=OHT)


def _build_nc(ET, W0, KE, ohoff, OHT):
    nc = bacc.Bacc(None, target_bir_lowering=False)

    xs = nc.dram_tensor("xs", [ET // 2 * P, 2 * H], BF, kind="ExternalInput")
    xtp = nc.dram_tensor("xtp", [NCH * P, 4 * P], BF, kind="ExternalInput")
    wf = nc.dram_tensor("wf", [NR2, W_COLS], F8, kind="ExternalInput")
    lw = nc.dram_tensor("lw", [KQ4, H], BF, kind="ExternalInput")
    etn = nc.dram_tensor("etn", [P, ET], I32, kind="ExternalInput")
    oh = nc.dram_tensor("oh", [OHT * P, P], F8, kind="ExternalInput")
    out = nc.dram_tensor("out", [N_PAD, H], BF, kind="ExternalOutput")

    with tile.TileContext(nc) as tc:
        with tc.tile_pool(name="const", bufs=1) as constp, \
             tc.tile_pool(name="s1", bufs=4) as s1, \
             tc.tile_pool(name="prodp", bufs=9) as prodp, \
             tc.tile_pool(name="t01p", bufs=4) as t01p, \
             tc.tile_pool(name="s2", bufs=6) as s2, \
             tc.tile_pool(name="psum", bufs=6, space="PSUM") as psp:

            # load the gather indices first (they gate the whole pipeline),
            # then the loop weights
            etn_sb = constp.tile([P, ET], I32, tag="etn")
            nc.sync.dma_start(out=etn_sb[:], in_=etn[:, :])
            lw_sb = []
            for q in range(4):
                t = constp.tile([P, H], BF, tag=f"lw{q}")
                nc.sync.dma_start(out=t[:], in_=lw[q * 128:(q + 1) * 128, :])
                lw_sb.append(t)

            prods = {}   # edge-tile idx -> list of NMM rhs views (+ tiles)

            def produce_pair(t):
                """Process edge tiles t and t+1 with pair-merged DVE ops."""
                xe2 = s1.tile([P, 2 * H], BF, tag="xe2")
                wg2 = s1.tile([P, 2 * W_COLS], BF, tag="wg2")
                m = t // 2
                nc.scalar.dma_start(out=xe2[:], in_=xs[m * P:(m + 1) * P, :])
                for k in range(2):
                    nc.gpsimd.indirect_dma_start(
                        out=wg2[:, k * W_COLS:(k + 1) * W_COLS],
                        out_offset=None, in_=wf[:, :],
                        in_offset=bass.IndirectOffsetOnAxis(
                            ap=etn_sb[:, t + k:t + k + 1], axis=0))
                # one full-width multiply for both tiles via the merged (k,i)
                # dim and a stride-0 broadcast over j:
                # prod[p, (k,i), j, b] = xe2[p, (k,i,b)] * wg2[p, (k,i,j,b)]
                pr2 = prodp.tile([P, 2 * W_COLS], BF, tag="pr2")
                xe_b = xe2[:].rearrange("p (ki u b) -> p ki u b", ki=2 * SUB, u=1) \
                             .to_broadcast([P, 2 * SUB, SUB, NB])
                nc.vector.tensor_tensor(
                    out=pr2[:].rearrange("p (ki j b) -> p ki j b", ki=2 * SUB, j=SUB),
                    in0=xe_b,
                    in1=wg2[:].rearrange("p (ki j b) -> p ki j b", ki=2 * SUB, j=SUB),
                    op=mybir.AluOpType.mult)
                # pair folds in two ops: one TT computes t0=s0+s1 and
                # t1=s2+s3 for both tiles via a [p, k, {0,2}, f] view; the
                # second folds t0+t1.
                pv = pr2[:].rearrange("p (k i f) -> p k i f", k=2, i=SUB)
                ev = pv[:, :, 0:4:2, :]   # slices i=0,2
                ov = pv[:, :, 1:4:2, :]   # slices i=1,3
                t01 = t01p.tile([P, 4 * H], BF, tag="t01")
                t01v = t01[:].rearrange("p (k i f) -> p k i f", k=2, i=2)
                nc.vector.tensor_tensor(out=t01v, in0=ev, in1=ov,
                                        op=mybir.AluOpType.add)
                t4 = t01[:].rearrange("p (k i f) -> p k i f", k=2, i=2)
                up = prodp.tile([P, 2 * H], BF, tag="up")
                nc.vector.tensor_tensor(
                    out=up[:].rearrange("p (k f) -> p k f", k=2),
                    in0=t4[:, :, 0, :], in1=t4[:, :, 1, :],
                    op=mybir.AluOpType.add)
                for k in range(2):
                    s4 = pr2[:, (k * SUB + 4) * H:(k * SUB + 5) * H]
                    prods[t + k] = [up[:, k * H:(k + 1) * H], s4]

            produced = 0
            for c in range(NCH):
                need = int(W0[c] + KE[c])
                while produced < need:
                    produce_pair(produced)
                    produced += 2
                ps = psp.tile([P, H], F32, tag="ps")
                ke = int(KE[c])
                ohsb = s2.tile([P, 7 * P], F8, tag="ohsb")
                o0 = int(ohoff[c]) * P
                nc.sync.dma_start(
                    out=ohsb[:, :ke * P],
                    in_=oh[o0:o0 + ke * P, :].rearrange("(p k) m -> p (k m)", p=P))
                xt = s2.tile([P, 4 * P], BF, tag="xt")
                nc.sync.dma_start(out=xt[:], in_=xtp[c * P:(c + 1) * P, :])
                # self-loop matmuls first: they only need xt, so the psum
                # group starts while late window tiles are still producing
                for q in range(4):
                    nc.tensor.matmul(out=ps[:], lhsT=xt[:, q * P:(q + 1) * P],
                                     rhs=lw_sb[q][:],
                                     start=(q == 0), stop=False)
                for kk in range(ke):
                    t = int(W0[c]) + kk
                    for rv in prods[t]:
                        last = (kk == ke - 1) and (rv is prods[t][-1])
                        nc.tensor.matmul(out=ps[:],
                                         lhsT=ohsb[:, kk * P:(kk + 1) * P],
                                         rhs=rv, start=False, stop=last)
                # PSUM columns are in (j,b) order; host un-permutes
                outt = s2.tile([P, H], BF, tag="outt")
                nc.scalar.activation(out=outt[:], in_=ps[:],
                                     func=mybir.ActivationFunctionType.Copy,
                                     scale=1.0 / (SW * SN))
                nc.sync.dma_start(out=out[c * P:(c + 1) * P, :], in_=outt[:])
                # drop window tiles no longer needed
                if c + 1 < NCH:
                    for t in [k for k in prods if k < int(W0[c + 1])]:
                        del prods[t]
    nc.finalize()
    return nc


def _run_layer(nc, plan, xp, xb, wfp, lwb, trace=False):
    """One RGCN-BDD layer (pre-bias, pre-activation) on 8 cores.

    xp: (i,b)-permuted features bf16 [NN, H]; xb: raw features bf16 [NN, H].
    """
    ET = plan['ET']
    in_maps = []
    for d in range(NDEV):
        # pair-swizzled per-edge features: row m*P+p holds tiles (2m, 2m+1)
        xg = xp[plan['srcl'][d]]
        xsd = np.ascontiguousarray(
            xg.reshape(ET // 2, 2, P, H).transpose(0, 2, 1, 3)
              .reshape(ET // 2 * P, 2 * H))
        # per-chunk contiguous transposed features: row c*P+p col q*P+m
        xt_full = np.zeros((KQ4, N_PAD), ml_dtypes.bfloat16)
        xt_full[:H, :NPD] = xb[d * NPD:(d + 1) * NPD].T
        xtpd = np.ascontiguousarray(
            xt_full.reshape(4, P, NCH, P).transpose(2, 1, 0, 3)
                   .reshape(NCH * P, 4 * P))
        in_maps.append({
            "xs": xsd, "xtp": xtpd, "wf": wfp, "lw": lwb,
            "etn": plan['etn'][d], "oh": plan['oh'][d],
        })
    res = run_bass_kernel_spmd(nc, in_maps, core_ids=list(range(NDEV)),
                               trace=trace)
    outp = np.empty((NN, H), np.float32)
    for d in range(NDEV):
        outp[d * NPD:(d + 1) * NPD] = res.results[d]["out"][:NPD].astype(np.float32)
    # device output columns are in (j,b) order; un-permute to (b,j)
    outp = np.ascontiguousarray(
        outp.reshape(NN, SUB, NB).transpose(0, 2, 1)).reshape(NN, H)
    return outp, res


def _pad_lw(lw):
    # permute output columns (b,j) -> (j,b) to match the message-path PSUM
    lwc = np.asarray(lw, np.float32).reshape(H, NB, SUB).transpose(0, 2, 1)
    lwp = np.zeros((KQ4, H), np.float32)
    lwp[:H] = lwc.reshape(H, H) * (SW * SN)
    return lwp.astype(ml_dtypes.bfloat16)


def _permute_w(W):
    # [r, b, i, j] -> [r, i, j, b] flattened, scaled, fp8_e3m4
    W = np.asarray(W, dtype=np.float32).reshape(NR2, NB, SUB, SUB)
    Wp = np.ascontiguousarray(
        W.transpose(0, 2, 3, 1).reshape(NR2, W_COLS)) * SW
    return np.clip(Wp, -F8MAX, F8MAX).astype(ml_dtypes.float8_e3m4)


def _permute_x(x):
    # [n, (b,i)] f32 -> [n, (i,b)] bf16
    return np.ascontiguousarray(
        x.reshape(-1, NB, SUB).transpose(0, 2, 1).reshape(-1, H)
    ).astype(ml_dtypes.bfloat16)


def kernel(nids, src, dst, etype, norm, emb, W1, loop_w1, bias1,
           W2, loop_w2, bias2, _trace=False, _times=None):
    key = "nc"
    if key not in _cache:
        plan = _plan(src, dst, etype, norm)
        nc = _build_nc(plan['ET'], plan['W0'], plan['KE'],
                       plan['ohoff'], plan['OHT'])
        _cache[key] = (plan, nc)
    plan, nc = _cache[key]

    x = np.asarray(emb, dtype=np.float32)[np.asarray(nids, dtype=np.int64)]
    h_pre, r1 = _run_layer(nc, plan, _permute_x(x),
                           x.astype(ml_dtypes.bfloat16),
                           _permute_w(W1), _pad_lw(loop_w1), trace=_trace)
    h = np.maximum(h_pre + np.asarray(bias1, dtype=np.float32)[None, :], 0.0)
    out_pre, r2 = _run_layer(nc, plan, _permute_x(h),
                             h.astype(ml_dtypes.bfloat16),
                             _permute_w(W2), _pad_lw(loop_w2), trace=_trace)
    out = out_pre + np.asarray(bias2, dtype=np.float32)[None, :]
    if _times is not None:
        _times.extend([r1, r2])
    return out


# revision 31
# speedup vs baseline: 1.0396x; 1.0389x over previous
"""RGCN-BDD link-predict layer kernel for 8 TRN2 NeuronCores.

Strategy: shard edges by destination-node slice (6250 nodes/device) so the
segment-sum is fully local; run the two RGCN layers as two launches of one
compiled single-layer NEFF, with host-side ReLU/bias between launches.

Per device, per layer (fused single pass):
  - edges are dst-sorted; per 128-node chunk the relevant edge tiles form a
    monotone sliding window, so per-edge product tiles stay SBUF-resident.
  - per 128-edge tile: load src features in (i,b)-permuted layout (bf16);
    indirect-gather per-edge block-diagonal weight rows from a host-permuted
    (i,j,b)-layout fp8_e3m4 table, cast to bf16 during the gather; a single
    full-width DVE multiply with a stride-0 broadcast view of the features
    forms all 2500 partial products (no on-chip expansion); DVE pairwise
    adds fold i-slices 0..3 into one.
  - per chunk: segment-sum via tensor-engine matmuls with host-built fp8
    one-hot matrices (entries carry the scaled edge norm), accumulated in
    PSUM together with the self-loop matmul; ACT copies PSUM out with the
    dequant scale.
"""
import sys
if '/opt/trn_rl_repo' not in sys.path:
    sys.path.insert(0, '/opt/trn_rl_repo')

import numpy as np
import ml_dtypes

import concourse.bass as bass
import concourse.bacc as bacc
import concourse.mybir as mybir
import concourse.tile as tile
from concourse.bass_utils import run_bass_kernel_spmd

# problem constants (hardcoded per spec)
NN = 50000      # num nodes
H = 500         # hidden dim
NB = 100        # num bases
SUB = 5         # block size
W_COLS = NB * SUB * SUB  # 2500
NR2 = 474       # num relations * 2
E = 100000      # num edges
NDEV = 8
P = 128
NPD = NN // NDEV          # 6250 nodes per device
NCH = (NPD + P - 1) // P  # 49 chunks
N_PAD = NCH * P           # 6272
KQ4 = 512  # K padded to 4*128 (zero rows beyond 500)

SW = 16.0   # weight-table fp8 scale
SN = 16.0   # one-hot (norm) fp8 scale
F8MAX = 15.5
# fold config: number of pairwise i-slice adds on DVE. NADD=3 folds slices
# 0..3 into one (tree t0=s0+s1, t1=s2+s3, u=t0+t1), leaving NMM=2 rhs
# slices {u, s4} for the tensor engine.
NADD = 3
NMM = 5 - NADD if NADD < 4 else 1
GP_T1 = False  # gpsimd folds collide with SWDGE descriptor generation

BF = mybir.dt.bfloat16
F8 = mybir.dt.float8e3
F32 = mybir.dt.float32
I32 = mybir.dt.int32

_cache = {}


def _plan(src, dst, etype, norm):
    """Host-side sharding plan; layer-invariant."""
    src = np.asarray(src).astype(np.int64)
    dst = np.asarray(dst).astype(np.int64)
    etype = np.asarray(etype).astype(np.int64)
    norm = np.asarray(norm).astype(np.float32).reshape(-1)

    dev_of = dst // NPD
    per = []
    for d in range(NDEV):
        sel = np.nonzero(dev_of == d)[0]
        dl = dst[sel] - d * NPD
        order = np.argsort(dl, kind='stable')
        el = sel[order]
        per.append((el, dl[order]))
    n_max = max(len(el) for el, _ in per)
    ET = (n_max + P - 1) // P

    # per-device padded src index list (for host-side pre-gather of xe rows)
    srcl = np.zeros((NDEV, ET * P), np.int64)

    # per-chunk union windows over edge tiles (same for all devices)
    W0 = np.zeros(NCH, np.int64)
    WEND = np.zeros(NCH, np.int64)
    for c in range(NCH):
        lo, hi = [], []
        for el, dl in per:
            e0 = np.searchsorted(dl, c * P, 'left')
            e1 = np.searchsorted(dl, (c + 1) * P, 'left')
            lo.append(e0 // P)
            hi.append((e1 + P - 1) // P if e1 > 0 else 0)
        W0[c] = min(lo)
        WEND[c] = max(max(hi), W0[c] + 1)
    WEND = np.minimum(WEND, ET)
    W0 = np.minimum(W0, WEND - 1)
    KE = (WEND - W0).astype(np.int64)
    OHT = int(KE.sum())           # total one-hot tiles
    ohoff = np.concatenate([[0], np.cumsum(KE)])[:NCH].astype(np.int64)

    # per-device static input arrays. oh is stored partition-major per chunk:
    # row ohoff[c]*P + p*KE[c] + kk holds edge-slot p of window tile kk, so
    # the per-chunk load is contiguous per partition.
    etn = np.zeros((NDEV, P, ET), np.int32)
    oh = np.zeros((NDEV, OHT * P, P), np.float32)
    for d in range(NDEV):
        el, dl = per[d]
        n_d = len(el)
        pad = ET * P - n_d
        srcl[d] = np.pad(src[el], (0, pad))
        etn[d] = np.pad(etype[el], (0, pad)).astype(np.int32).reshape(ET, P).T
        nr = norm[el]
        for c in range(NCH):
            for kk in range(KE[c]):
                g0 = (W0[c] + kk) * P
                rows = np.arange(g0, g0 + P)
                valid = rows < n_d
                m = dl[rows[valid]] - c * P
                ok = (m >= 0) & (m < P)
                j = np.nonzero(valid)[0][ok]
                oh[d, ohoff[c] * P + j * KE[c] + kk, m[ok]] = nr[rows[valid]][ok]

    oh8 = np.clip(oh * SN, -F8MAX, F8MAX).astype(ml_dtypes.float8_e3m4)
    return dict(ET=ET, srcl=srcl, etn=etn,
                oh=oh8, W0=W0, KE=KE, ohoff=ohoff, OHT=OHT)


def _build_nc(ET, W0, KE, ohoff, OHT):
    nc = bacc.Bacc(None, target_bir_lowering=False)

    xs = nc.dram_tensor("xs", [ET // 2 * P, 2 * H], BF, kind="ExternalInput")
    xtp = nc.dram_tensor("xtp", [NCH * P, 4 * P], BF, kind="ExternalInput")
    wf = nc.dram_tensor("wf", [NR2, W_COLS], F8, kind="ExternalInput")
    lw = nc.dram_tensor("lw", [KQ4, H], BF, kind="ExternalInput")
    etn = nc.dram_tensor("etn", [P, ET], I32, kind="ExternalInput")
    oh = nc.dram_tensor("oh", [OHT * P, P], F8, kind="ExternalInput")
    out = nc.dram_tensor("out", [N_PAD, H], BF, kind="ExternalOutput")

    with tile.TileContext(nc) as tc:
        with tc.tile_pool(name="const", bufs=1) as constp, \
             tc.tile_pool(name="s1", bufs=4) as s1, \
             tc.tile_pool(name="prodp", bufs=9) as prodp, \
             tc.tile_pool(name="s2", bufs=4) as s2, \
             tc.tile_pool(name="psum", bufs=6, space="PSUM") as psp:

            # load the gather indices first (they gate the whole pipeline),
            # then the loop weights
            etn_sb = constp.tile([P, ET], I32, tag="etn")
            nc.sync.dma_start(out=etn_sb[:], in_=etn[:, :])
            lw_sb = []
            for q in range(4):
                t = constp.tile([P, H], BF, tag=f"lw{q}")
                nc.sync.dma_start(out=t[:], in_=lw[q * 128:(q + 1) * 128, :])
                lw_sb.append(t)

            prods = {}   # edge-tile idx -> list of NMM rhs views (+ tiles)

            def produce_pair(t):
                """Process edge tiles t and t+1 with pair-merged DVE ops."""
                xe2 = s1.tile([P, 2 * H], BF, tag="xe2")
                wg2 = s1.tile([P, 2 * W_COLS], BF, tag="wg2")
                m = t // 2
                nc.scalar.dma_start(out=xe2[:], in_=xs[m * P:(m + 1) * P, :])
                for k in range(2):
                    nc.gpsimd.indirect_dma_start(
                        out=wg2[:, k * W_COLS:(k + 1) * W_COLS],
                        out_offset=None, in_=wf[:, :],
                        in_offset=bass.IndirectOffsetOnAxis(
                            ap=etn_sb[:, t + k:t + k + 1], axis=0))
                # one full-width multiply for both tiles via the merged (k,i)
                # dim and a stride-0 broadcast over j:
                # prod[p, (k,i), j, b] = xe2[p, (k,i,b)] * wg2[p, (k,i,j,b)]
                pr2 = prodp.tile([P, 2 * W_COLS], BF, tag="pr2")
                xe_b = xe2[:].rearrange("p (ki u b) -> p ki u b", ki=2 * SUB, u=1) \
                             .to_broadcast([P, 2 * SUB, SUB, NB])
                nc.vector.tensor_tensor(
                    out=pr2[:].rearrange("p (ki j b) -> p ki j b", ki=2 * SUB, j=SUB),
                    in0=xe_b,
                    in1=wg2[:].rearrange("p (ki j b) -> p ki j b", ki=2 * SUB, j=SUB),
                    op=mybir.AluOpType.mult)
                # pair folds in two ops: one TT computes t0=s0+s1 and
                # t1=s2+s3 for both tiles via a [p, k, {0,2}, f] view; the
                # second folds t0+t1.
                pv = pr2[:].rearrange("p (k i f) -> p k i f", k=2, i=SUB)
                ev = pv[:, :, 0:4:2, :]   # slices i=0,2
                ov = pv[:, :, 1:4:2, :]   # slices i=1,3
                t01 = s1.tile([P, 4 * H], BF, tag="t01")
                t01v = t01[:].rearrange("p (k i f) -> p k i f", k=2, i=2)
                nc.vector.tensor_tensor(out=t01v, in0=ev, in1=ov,
                                        op=mybir.AluOpType.add)
                t4 = t01[:].rearrange("p (k i f) -> p k i f", k=2, i=2)
                up = prodp.tile([P, 2 * H], BF, tag="up")
                nc.vector.tensor_tensor(
                    out=up[:].rearrange("p (k f) -> p k f", k=2),
                    in0=t4[:, :, 0, :], in1=t4[:, :, 1, :],
                    op=mybir.AluOpType.add)
                for k in range(2):
                    s4 = pr2[:, (k * SUB + 4) * H:(k * SUB + 5) * H]
                    prods[t + k] = [up[:, k * H:(k + 1) * H], s4]

            produced = 0
            for c in range(NCH):
                need = int(W0[c] + KE[c])
                while produced < need:
                    produce_pair(produced)
                    produced += 2
                ps = psp.tile([P, H], F32, tag="ps")
                ke = int(KE[c])
                ohsb = s2.tile([P, 7 * P], F8, tag="ohsb")
                o0 = int(ohoff[c]) * P
                nc.sync.dma_start(
                    out=ohsb[:, :ke * P],
                    in_=oh[o0:o0 + ke * P, :].rearrange("(p k) m -> p (k m)", p=P))
                xt = s2.tile([P, 4 * P], BF, tag="xt")
                nc.sync.dma_start(out=xt[:], in_=xtp[c * P:(c + 1) * P, :])
                # self-loop matmuls first: they only need xt, so the psum
                # group starts while late window tiles are still producing
                for q in range(4):
                    nc.tensor.matmul(out=ps[:], lhsT=xt[:, q * P:(q + 1) * P],
                                     rhs=lw_sb[q][:],
                                     start=(q == 0), stop=False)
                for kk in range(ke):
                    t = int(W0[c]) + kk
                    for rv in prods[t]:
                        last = (kk == ke - 1) and (rv is prods[t][-1])
                        nc.tensor.matmul(out=ps[:],
                                         lhsT=ohsb[:, kk * P:(kk + 1) * P],
                                         rhs=rv, start=False, stop=last)
                # PSUM columns are in (j,b) order; host un-permutes
                outt = s2.tile([P, H], BF, tag="outt")
                nc.scalar.activation(out=outt[:], in_=ps[:],
                                     func=mybir.ActivationFunctionType.Copy,
                                     scale=1.0 / (SW * SN))
                nc.sync.dma_start(out=out[c * P:(c + 1) * P, :], in_=outt[:])
                # drop window tiles no longer needed
                if c + 1 < NCH:
                    for t in [k for k in prods if k < int(W0[c + 1])]:
                        del prods[t]
    nc.finalize()
    return nc


def _run_layer(nc, plan, xp, xb, wfp, lwb, trace=False):
    """One RGCN-BDD layer (pre-bias, pre-activation) on 8 cores.

    xp: (i,b)-permuted features bf16 [NN, H]; xb: raw features bf16 [NN, H].
    """
    ET = plan['ET']
    in_maps = []
    for d in range(NDEV):
        # pair-swizzled per-edge features: row m*P+p holds tiles (2m, 2m+1)
        xg = xp[plan['srcl'][d]]
        xsd = np.ascontiguousarray(
            xg.reshape(ET // 2, 2, P, H).transpose(0, 2, 1, 3)
              .reshape(ET // 2 * P, 2 * H))
        # per-chunk contiguous transposed features: row c*P+p col q*P+m
        xt_full = np.zeros((KQ4, N_PAD), ml_dtypes.bfloat16)
        xt_full[:H, :NPD] = xb[d * NPD:(d + 1) * NPD].T
        xtpd = np.ascontiguousarray(
            xt_full.reshape(4, P, NCH, P).transpose(2, 1, 0, 3)
                   .reshape(NCH * P, 4 * P))
        in_maps.append({
            "xs": xsd, "xtp": xtpd, "wf": wfp, "lw": lwb,
            "etn": plan['etn'][d], "oh": plan['oh'][d],
        })
    res = run_bass_kernel_spmd(nc, in_maps, core_ids=list(range(NDEV)),
                               trace=trace)
    outp = np.empty((NN, H), np.float32)
    for d in range(NDEV):
        outp[d * NPD:(d + 1) * NPD] = res.results[d]["out"][:NPD].astype(np.float32)
    # device output columns are in (j,b) order; un-permute to (b,j)
    outp = np.ascontiguousarray(
        outp.reshape(NN, SUB, NB).transpose(0, 2, 1)).reshape(NN, H)
    return outp, res


def _pad_lw(lw):
    # permute output columns (b,j) -> (j,b) to match the message-path PSUM
    lwc = np.asarray(lw, np.float32).reshape(H, NB, SUB).transpose(0, 2, 1)
    lwp = np.zeros((KQ4, H), np.float32)
    lwp[:H] = lwc.reshape(H, H) * (SW * SN)
    return lwp.astype(ml_dtypes.bfloat16)


def _permute_w(W):
    # [r, b, i, j] -> [r, i, j, b] flattened, scaled, fp8_e3m4
    W = np.asarray(W, dtype=np.float32).reshape(NR2, NB, SUB, SUB)
    Wp = np.ascontiguousarray(
        W.transpose(0, 2, 3, 1).reshape(NR2, W_COLS)) * SW
    return np.clip(Wp, -F8MAX, F8MAX).astype(ml_dtypes.float8_e3m4)


def _permute_x(x):
    # [n, (b,i)] f32 -> [n, (i,b)] bf16
    return np.ascontiguousarray(
        x.reshape(-1, NB, SUB).transpose(0, 2, 1).reshape(-1, H)
    ).astype(ml_dtypes.bfloat16)


def kernel(nids, src, dst, etype, norm, emb, W1, loop_w1, bias1,
           W2, loop_w2, bias2, _trace=False, _times=None):
    key = "nc"
    if key not in _cache:
        plan = _plan(src, dst, etype, norm)
        nc = _build_nc(plan['ET'], plan['W0'], plan['KE'],
                       plan['ohoff'], plan['OHT'])
        _cache[key] = (plan, nc)
    plan, nc = _cache[key]

    x = np.asarray(emb, dtype=np.float32)[np.asarray(nids, dtype=np.int64)]
    h_pre, r1 = _run_layer(nc, plan, _permute_x(x),
                           x.astype(ml_dtypes.bfloat16),
                           _permute_w(W1), _pad_lw(loop_w1), trace=_trace)
    h = np.maximum(h_pre + np.asarray(bias1, dtype=np.float32)[None, :], 0.0)
    out_pre, r2 = _run_layer(nc, plan, _permute_x(h),
                             h.astype(ml_dtypes.bfloat16),
                             _permute_w(W2), _pad_lw(loop_w2), trace=_trace)
    out = out_pre + np.asarray(bias2, dtype=np.float32)[None, :]
    if _times is not None:
        _times.extend([r1, r2])
    return out
